# revision 1
# baseline (speedup 1.0000x reference)
"""Trainium2 Bass kernel for BatchedACE (LSH-softmax linear attention).

Math (per fused sequence n of N = M*B*H = 32):
  probs(X)[t, l, r] = softmax_r( tanh(X @ planes)/sqrt(dk) @ protos )
  A = cumsum_t(probsK)                      [T, L, R]
  S_t = cumsum_t(probsK x V outer)          [L, R, dk]
  out[t] = sum_{l,r} probsQ[t,l,r] * S_t[l,r,:] / (A[t,l,r] + 1e-6)

Key facts exploited on-chip:
  * L*R = 128 = partition dim; everything runs in [lr, t] layout.
  * chunked linear attention: per 128-chunk, out = mask(P^T Qp)^T V + Qp^T S
  * A-cumsum is a native DVE tensor_tensor_scan along the free dim.
  * |logits| <= 0.5 so softmax needs no max-subtraction.

Sharding: N=32 sequences split 4-per-core across 8 NeuronCores; no
cross-core communication.
"""
import numpy as np
import ml_dtypes
from contextlib import ExitStack

import concourse.bass as bass
import concourse.tile as tile
from concourse import bacc, mybir
from concourse.bass_utils import run_bass_kernel_spmd

BF16 = ml_dtypes.bfloat16
BF = mybir.dt.bfloat16
F32 = mybir.dt.float32
Alu = mybir.AluOpType
Act = mybir.ActivationFunctionType

M_ENS, B_SZ, T_LEN, H_HEADS, D_K = 2, 2, 512, 8, 64
K_BITS, L_TABLES, R_CORNERS = 4, 8, 16
N_TOTAL = M_ENS * B_SZ * H_HEADS          # 32
NCORES = 8
SEQ = N_TOTAL // NCORES                   # 4 sequences per core
CH = 128                                  # chunk length (partition dim)
NCH = T_LEN // CH                         # 4 chunks
LR = L_TABLES * R_CORNERS                 # 128
LK = L_TABLES * K_BITS                    # 32
EPS = 1e-6

_CACHE = {}


def _build_module(n_iters=1):
    """n_iters>1 wraps the body in a hardware For_i loop (timing builds)."""
    nc = bacc.Bacc("TRN2", target_bir_lowering=False, debug=False,
                   num_devices=NCORES)

    # per-core inputs
    kt_d = nc.dram_tensor("kt", [D_K, SEQ * T_LEN], BF, kind="ExternalInput").ap()
    qt_d = nc.dram_tensor("qt", [D_K, SEQ * T_LEN], BF, kind="ExternalInput").ap()
    v_d = nc.dram_tensor("v", [CH, SEQ * NCH * D_K], BF, kind="ExternalInput").ap()
    pw_d = nc.dram_tensor("pw", [128, LR + LK], BF, kind="ExternalInput").ap()
    out_d = nc.dram_tensor("out_t", [SEQ, D_K, T_LEN], F32, kind="ExternalOutput").ap()

    # structural constants, packed into one inline-const DMA:
    # [mask4 f32 | bf16 section bit-packed into f32 words]
    bones4_np = np.zeros((128, LR), dtype=np.float32)
    for s in range(4):
        for j in range(L_TABLES):
            bones4_np[32 * s + j, j * R_CORNERS:(j + 1) * R_CORNERS] = 1.0
    mask_np = (np.arange(CH)[:, None] <= np.arange(CH)[None, :]).astype(np.float32)
    mask4_np = np.tile(mask_np, (1, SEQ))
    ones32_np = (np.arange(LR)[:, None] // R_CORNERS ==
                 (np.arange(4 * L_TABLES)[None, :] % L_TABLES))
    bf_sec = np.concatenate([
        bones4_np.astype(BF16),                                     # 128 cols
        np.eye(128, dtype=BF16),                                    # 128 cols
        ones32_np.astype(BF16),                                     # 32 cols
    ], axis=1)                                                      # [128, 288] bf16
    bf_as_f32 = bf_sec.view(np.uint16).reshape(128, 144, 2)
    bf_words = (bf_as_f32[:, :, 0].astype(np.uint32) |
                (bf_as_f32[:, :, 1].astype(np.uint32) << 16)).view(np.float32)
    blob_np = np.concatenate([mask4_np, bf_words], axis=1)
    blob_c = nc.inline_tensor(blob_np, name="blob_c")

    with tile.TileContext(nc) as tc:
        with ExitStack() as ctx:
            cp = ctx.enter_context(tc.tile_pool(name="consts", bufs=1))
            sp = ctx.enter_context(tc.tile_pool(name="sb", bufs=1))
            lp = ctx.enter_context(tc.tile_pool(name="loop", bufs=5))
            plog = ctx.enter_context(tc.tile_pool(name="plog", bufs=1, space="PSUM"))
            pw = ctx.enter_context(tc.tile_pool(name="pw", bufs=6, space="PSUM"))
            if n_iters > 1:
                ctx.enter_context(tc.For_i(0, n_iters, 1, hint_engines=(mybir.EngineType.PE,)))

            pw_sb = cp.tile([128, LR + LK], BF)
            nc.sync.dma_start(pw_sb[:], pw_d)
            kt_sb = sp.tile([D_K, SEQ * T_LEN], BF)
            nc.sync.dma_start(kt_sb[:, 0:2 * T_LEN], kt_d[:, 0:2 * T_LEN])
            nc.sync.dma_start(kt_sb[:, 2 * T_LEN:], kt_d[:, 2 * T_LEN:])
            qt_sb = sp.tile([D_K, SEQ * T_LEN], BF)
            nc.sync.dma_start(qt_sb[:], qt_d)
            v_sb = sp.tile([CH, SEQ * NCH * D_K], BF)
            nc.sync.dma_start(v_sb[:], v_d)
            blob_sb = cp.tile([128, SEQ * CH + 144], F32)
            nc.sync.dma_start(blob_sb[:], blob_c.ap())

            w4_sb = pw_sb[:, 0:LR]
            planes_sb = pw_sb[0:D_K, LR:LR + LK]
            mask4_sb = blob_sb[:, 0:SEQ * CH]
            bf_view = blob_sb[:, SEQ * CH:SEQ * CH + 144].bitcast(BF)
            bones4_sb = bf_view[:, 0:128]
            ident_sb = bf_view[:, 128:256]
            ones32_sb = bf_view[:, 256:288]

            def S(s):
                return slice(T_LEN * s, T_LEN * (s + 1))

            # ---- probs pipelines: full K chain first, then Q ----
            xt = {"k": kt_sb, "q": qt_sb}
            dst = {}
            dst["k"] = sp.tile([128, SEQ * T_LEN], BF, tag="ptk", name="ptk")
            dst["q"] = sp.tile([128, SEQ * T_LEN], BF, tag="qeq", name="qeq")

            def probs_chain(x):
                proj_ps = pw.tile([128, T_LEN], F32, tag="w", name=f"proj{x}")
                for s in range(SEQ):
                    nc.tensor.matmul(proj_ps[32 * s:32 * s + 32, :],
                                     planes_sb, xt[x][:, S(s)],
                                     start=True, stop=True,
                                     tile_position=(0, 32 * s))
                tanh_sb = lp.tile([128, T_LEN], BF, tag=f"tanh{x}", name=f"tanh{x}")
                nc.scalar.activation(tanh_sb[:], proj_ps[:], Act.Tanh)

                e_sb = sp.tile([128, SEQ * T_LEN], BF, tag=f"e{x}", name=f"e{x}")
                sums_ps = pw.tile([128, T_LEN], F32, tag="w", name=f"sums{x}")
                for s in range(SEQ):
                    logit_ps = pw.tile([128, T_LEN], F32, tag="w",
                                       name=f"log{x}{s}")
                    nc.tensor.matmul(logit_ps[:],
                                     w4_sb[32 * s:32 * s + 32, :],
                                     tanh_sb[32 * s:32 * s + 32, :],
                                     start=True, stop=True,
                                     tile_position=(32 * s, 0))
                    nc.scalar.activation(e_sb[:, S(s)], logit_ps[:], Act.Exp)
                    nc.tensor.matmul(sums_ps[32 * s:32 * s + 32, :],
                                     ones32_sb, e_sb[:, S(s)],
                                     start=True, stop=True,
                                     tile_position=(0, 32 * s))
                # reciprocal of softmax sums -> bf16 -> broadcast over the
                # 16 corners via a block-ones matmul, then normalize on DVE
                recip_f = lp.tile([128, T_LEN], F32, tag=f"recipf{x}",
                                  name=f"recipf{x}")
                recip_b = lp.tile([128, T_LEN], BF, tag=f"recip{x}",
                                  name=f"recip{x}")
                nc.vector.reciprocal_approx_fast(recip_f[:], sums_ps[:])
                nc.scalar.copy(recip_b[:], recip_f[:])
                for h in range(2):
                    b = plog.tile([128, 2 * T_LEN], F32, tag="log",
                                  name=f"bc{x}{h}")
                    for i in range(2):
                        s = 2 * h + i
                        nc.tensor.matmul(b[:, T_LEN * i:T_LEN * (i + 1)],
                                         bones4_sb[32 * s:32 * s + 8, :],
                                         recip_b[32 * s:32 * s + 8, :],
                                         start=True, stop=True,
                                         tile_position=(32 * s, 0))
                    cols = slice(T_LEN * 2 * h, T_LEN * 2 * (h + 1))
                    nc.vector.tensor_mul(dst[x][:, cols], e_sb[:, cols], b[:])
            probs_chain("k")
            pt_sb = dst["k"]
            probs_chain("q")
            qe_sb = dst["q"]

            # ---- chunked attention ----
            def tsl(s, c):
                return slice(T_LEN * s + CH * c, T_LEN * s + CH * (c + 1))

            def vsl(s, c):
                return slice(D_K * (s * NCH + c), D_K * (s * NCH + c + 1))

            # A = cumsum(P) + eps, then Qp (per seq, all on DVE)
            a_sb = sp.tile([128, SEQ * T_LEN], F32)
            ra_sb = sp.tile([128, SEQ * T_LEN], F32)
            qp_sb = sp.tile([128, SEQ * T_LEN], BF)
            for s in range(SEQ):
                nc.vector.tensor_tensor_scan(a_sb[:, S(s)], pt_sb[:, S(s)],
                                             pt_sb[:, S(s)], EPS,
                                             Alu.add, Alu.bypass)
                nc.vector.reciprocal_approx_fast(ra_sb[:, S(s)], a_sb[:, S(s)])
                nc.vector.tensor_mul(qp_sb[:, S(s)], qe_sb[:, S(s)],
                                     ra_sb[:, S(s)])

            # intra-chunk quadratic term + P transposes
            gm_sb, pn_sb = {}, {}
            for c in range(NCH):
                gt_ps = pw.tile([CH, SEQ * CH], F32, tag="w")
                for s in range(SEQ):
                    nc.tensor.matmul(gt_ps[:, CH * s:CH * (s + 1)],
                                     pt_sb[:, tsl(s, c)], qp_sb[:, tsl(s, c)],
                                     start=True, stop=True)
                gm_sb[c] = lp.tile([CH, SEQ * CH], BF, tag="gm", name=f"gm{c}")
                nc.vector.tensor_mul(gm_sb[c][:], gt_ps[:], mask4_sb[:])

                if c < NCH - 1:
                    tr_ps = pw.tile([CH, SEQ * CH], BF, tag="w")
                    for s in range(SEQ):
                        nc.tensor.transpose(tr_ps[:, CH * s:CH * (s + 1)],
                                            pt_sb[:, tsl(s, c)], ident_sb[:])
                    pn_sb[c] = lp.tile([CH, SEQ * CH], BF, tag="pn", name=f"pn{c}")
                    nc.scalar.copy(pn_sb[c][:], tr_ps[:])

            s_tiles = []
            s_prev = None
            for c in range(NCH - 1):
                ds_ps = pw.tile([LR, SEQ * D_K], F32, tag="w", name=f"dsp{c}")
                for s in range(SEQ):
                    nc.tensor.matmul(ds_ps[:, D_K * s:D_K * (s + 1)],
                                     pn_sb[c][:, CH * s:CH * (s + 1)],
                                     v_sb[:, vsl(s, c)],
                                     start=True, stop=True)
                s_new = sp.tile([LR, SEQ * D_K], BF, tag=f"state{c}",
                                name=f"state{c}")
                if s_prev is None:
                    nc.scalar.copy(s_new[:], ds_ps[:])
                else:
                    nc.vector.tensor_add(s_new[:], ds_ps[:], s_prev[:])
                s_tiles.append(s_new)
                s_prev = s_new

            for c in range(NCH):
                out_ps = pw.tile([D_K, SEQ * CH], F32, tag="w")
                for s in range(SEQ):
                    nc.tensor.matmul(out_ps[:, CH * s:CH * (s + 1)],
                                     v_sb[:, vsl(s, c)],
                                     gm_sb[c][:, CH * s:CH * (s + 1)],
                                     start=True, stop=(c == 0))
                    if c > 0:
                        nc.tensor.matmul(out_ps[:, CH * s:CH * (s + 1)],
                                         s_tiles[c - 1][:, D_K * s:D_K * (s + 1)],
                                         qp_sb[:, tsl(s, c)],
                                         start=False, stop=True)
                out_sb = lp.tile([D_K, SEQ * CH], F32, tag="osb")
                nc.scalar.copy(out_sb[:], out_ps[:])
                nc.sync.dma_start(
                    out_d[:, :, CH * c:CH * (c + 1)].rearrange("s d t -> d s t"),
                    out_sb[:].rearrange("d (s t) -> d s t", s=SEQ))

    nc.compile()
    return nc


def _host_prep(Khf, Vhf, Qhf, planes_T, protos_T):
    """Fold + transpose + quantize inputs; build per-core in_maps."""
    Khf = np.asarray(Khf, dtype=np.float32)
    Vhf = np.asarray(Vhf, dtype=np.float32)
    Qhf = np.asarray(Qhf, dtype=np.float32)
    planes_T = np.asarray(planes_T, dtype=np.float32)
    protos_T = np.asarray(protos_T, dtype=np.float32)
    scale = np.sqrt(np.float32(D_K))

    def fold(x):
        return np.transpose(x, (0, 1, 3, 2, 4)).reshape(N_TOTAL, T_LEN, D_K)

    K2, Q2, V2 = fold(Khf), fold(Qhf), fold(Vhf)
    KT = np.ascontiguousarray(np.transpose(K2, (0, 2, 1))).astype(BF16)  # [N, dk, T]
    QT = np.ascontiguousarray(np.transpose(Q2, (0, 2, 1))).astype(BF16)
    V4 = V2.reshape(N_TOTAL, NCH, CH, D_K)

    w4 = np.zeros((128, LR), dtype=np.float32)
    wblk = np.zeros((LK, LR), dtype=np.float32)
    for l in range(L_TABLES):
        wblk[l * K_BITS:(l + 1) * K_BITS, l * R_CORNERS:(l + 1) * R_CORNERS] = \
            protos_T / scale
    for s in range(4):
        w4[32 * s:32 * s + 32, :] = wblk
    pw = np.zeros((128, LR + LK), dtype=BF16)
    pw[:, 0:LR] = w4.astype(BF16)
    pw[0:D_K, LR:LR + LK] = planes_T.astype(BF16)

    in_maps = []
    for core in range(NCORES):
        ns = slice(SEQ * core, SEQ * (core + 1))
        ktc = np.ascontiguousarray(KT[ns]).reshape(SEQ, D_K, T_LEN)
        qtc = np.ascontiguousarray(QT[ns]).reshape(SEQ, D_K, T_LEN)
        vc = np.ascontiguousarray(
            np.transpose(V4[ns], (2, 0, 1, 3))).astype(BF16)  # [128, seq, ch, dk]
        in_maps.append({
            "kt": np.ascontiguousarray(np.transpose(ktc, (1, 0, 2))).reshape(D_K, SEQ * T_LEN),
            "qt": np.ascontiguousarray(np.transpose(qtc, (1, 0, 2))).reshape(D_K, SEQ * T_LEN),
            "v": vc.reshape(CH, SEQ * NCH * D_K),
            "pw": pw,
        })
    return in_maps


def kernel(Khf, Vhf, Qhf, planes_T, protos_T, _results_hook=None):
    if "nc" not in _CACHE:
        _CACHE["nc"] = _build_module()
    nc = _CACHE["nc"]
    in_maps = _host_prep(Khf, Vhf, Qhf, planes_T, protos_T)
    res = run_bass_kernel_spmd(nc, in_maps, list(range(NCORES)))
    if _results_hook is not None:
        _results_hook(res)
    out = np.empty((N_TOTAL, T_LEN, D_K), dtype=np.float32)
    for core in range(NCORES):
        out_t = res.results[core]["out_t"]          # [SEQ, dk, T]
        out[SEQ * core:SEQ * (core + 1)] = np.transpose(out_t, (0, 2, 1))
    return np.ascontiguousarray(
        out.reshape(M_ENS, B_SZ, H_HEADS, T_LEN, D_K).transpose(0, 1, 3, 2, 4))



# revision 5
# speedup vs baseline: 1.4447x; 1.4447x over previous
"""Trainium2 Bass kernel for BatchedACE (LSH-softmax linear attention), v2.

Math (per fused sequence n of N = M*B*H = 32):
  probs(X)[t, l, r] = softmax_r( tanh(X @ planes)/sqrt(dk) @ protos )
  A = cumsum_t(probsK) + eps                [T, L, R]
  S_t = cumsum_t(probsK x V outer)          [L, R, dk]
  out[t] = sum_{l,r} probsQ[t,l,r] * S_t[l,r,:] / A[t,l,r]

v2 key trick: the per-table softmax over the 2^K hypercube corners has an
ANALYTIC partition function:
  Z[l,t] = prod_k 2*cosh(tanh_k/8)  =>  logZ = 4*log2 + sum_k tanh_k^2/128
(+O(tau^4/49152) ~ 8e-5 rel).  So probs = exp(logits - logZ) needs NO
softmax-denominator machinery: the tau^2 sum rides as extra contraction rows
in the logits matmul, and -4log2 folds into the exp bias.

Layout: seq-pair contraction packing: kt2 [128=(a,dk), 1024=(h,t)] lets one
proj matmul compute 2 sequences at once AND duplicate tanh rows into
partitions 64-127 (extra stationary columns are free), where they are
squared in place to feed the logit matmul's 128-deep contraction.

Sharding: N=32 sequences, 4 per core across 8 NeuronCores; no cross-core
communication.
"""
import math
import numpy as np
import ml_dtypes
from contextlib import ExitStack

import concourse.bass as bass
import concourse.tile as tile
from concourse import bacc, mybir
from concourse.bass_utils import run_bass_kernel_spmd

BF16 = ml_dtypes.bfloat16
BF = mybir.dt.bfloat16
F32 = mybir.dt.float32
Alu = mybir.AluOpType
Act = mybir.ActivationFunctionType

M_ENS, B_SZ, T_LEN, H_HEADS, D_K = 2, 2, 512, 8, 64
K_BITS, L_TABLES, R_CORNERS = 4, 8, 16
N_TOTAL = M_ENS * B_SZ * H_HEADS          # 32
NCORES = 8
SEQ = N_TOTAL // NCORES                   # 4 sequences per core
CH = 128                                  # chunk length (partition dim)
NCH = T_LEN // CH                         # 4 chunks
LR = L_TABLES * R_CORNERS                 # 128
LK = L_TABLES * K_BITS                    # 32
EPS = 1e-6
NLOG2x4 = -4.0 * math.log(2.0)

_CACHE = {}

# engine assignment knobs (tuned against TimelineSim)
CFG = {
    "scan": "vector",      # per-seq cumsum of probsK
    "recip": "vector",     # 1/A
    "qp": "vector",        # probsQ * ra
    "sq": "vector",        # tau^2 in place
    "gm": ["vector", "gpsimd", "vector", "gpsimd"],   # mask mul per chunk
    "pn": ["vector", "gpsimd", "vector"],             # P^T PSUM->SBUF copies
    "scopy": "vector",     # state snapshots PSUM->SBUF
}


def _eng(nc, name):
    return {"vector": nc.vector, "gpsimd": nc.gpsimd, "scalar": nc.scalar}[name]


def _build_module(n_iters=1, dbg=False):
    """n_iters>1 wraps the body in a hardware For_i loop (timing builds)."""
    nc = bacc.Bacc("TRN2", target_bir_lowering=False, debug=False,
                   num_devices=NCORES)
    dbg_d = {}
    if dbg:
        for nm in ("P", "Qe", "A", "qp", "gm0", "gm1", "pn0", "st0", "st1"):
            w = SEQ * T_LEN if nm in ("P", "Qe", "A", "qp") else \
                (SEQ * CH if nm.startswith(("gm", "pn")) else SEQ * D_K)
            dt = F32 if nm == "A" else BF
            dbg_d[nm] = nc.dram_tensor(f"dbg_{nm}", [128, w], dt,
                                       kind="ExternalOutput").ap()

    kt2_d = nc.dram_tensor("kt2", [128, 2 * T_LEN], BF, kind="ExternalInput").ap()
    qt2_d = nc.dram_tensor("qt2", [128, 2 * T_LEN], BF, kind="ExternalInput").ap()
    v_d = nc.dram_tensor("v", [CH, SEQ * NCH * D_K], BF, kind="ExternalInput").ap()
    cw_d = nc.dram_tensor("cw", [128, 642], BF, kind="ExternalInput").ap()
    out_d = nc.dram_tensor("out_t", [SEQ, D_K, T_LEN], F32, kind="ExternalOutput").ap()

    with tile.TileContext(nc) as tc:
        with ExitStack() as ctx:
            cp = ctx.enter_context(tc.tile_pool(name="consts", bufs=1))
            sp = ctx.enter_context(tc.tile_pool(name="sb", bufs=1))
            pw = ctx.enter_context(tc.tile_pool(name="pw", bufs=6, space="PSUM"))
            pst = ctx.enter_context(tc.tile_pool(name="pst", bufs=1, space="PSUM"))
            if n_iters > 1:
                ctx.enter_context(tc.For_i(0, n_iters, 1,
                                           hint_engines=(mybir.EngineType.PE,)))

            cw_sb = cp.tile([128, 642], BF)
            nc.sync.dma_start(cw_sb[:], cw_d)
            kt2_sb = sp.tile([128, 2 * T_LEN], BF)
            nc.sync.dma_start(kt2_sb[:], kt2_d)
            qt2_sb = sp.tile([128, 2 * T_LEN], BF)
            nc.sync.dma_start(qt2_sb[:], qt2_d)
            v_sb = sp.tile([CH, SEQ * NCH * D_K], BF)
            nc.sync.dma_start(v_sb[:], v_d)

            wproj = cw_sb[:, 0:128]
            wlog = [cw_sb[:, 128:256], cw_sb[:, 256:384]]
            ident = cw_sb[:, 384:512]
            mask1 = cw_sb[:, 512:640]
            ebias = cw_sb[:, 640:642].bitcast(F32)      # [128, 1] = -4*log2

            def S(s):
                return slice(T_LEN * s, T_LEN * (s + 1))

            def tsl(s, c):
                return slice(T_LEN * s + CH * c, T_LEN * s + CH * (c + 1))

            def vsl(s, c):
                return slice(D_K * (s * NCH + c), D_K * (s * NCH + c + 1))

            # ---- probs stage: P = probsK, Qe = probsQ (both normalized) ----
            xt = {"k": kt2_sb, "q": qt2_sb}
            dst = {"k": sp.tile([128, SEQ * T_LEN], BF, name="pk"),
                   "q": sp.tile([128, SEQ * T_LEN], BF, name="pq")}

            for x in ("k", "q"):
                ts_h = []
                for h in range(2):
                    pp = pw.tile([128, T_LEN], F32, tag="w", name=f"pp{x}{h}")
                    nc.tensor.matmul(pp[:], wproj, xt[x][:, S(h)],
                                     start=True, stop=True)
                    ts = sp.tile([128, T_LEN], BF, name=f"ts{x}{h}")
                    nc.scalar.activation(ts[:], pp[:], Act.Tanh)
                    _eng(nc, CFG["sq"]).tensor_mul(
                        ts[64:128, :], ts[64:128, :], ts[64:128, :])
                    ts_h.append(ts)
                for s in range(SEQ):
                    h, a = s // 2, s % 2
                    lg = pw.tile([128, T_LEN], F32, tag="w", name=f"lg{x}{s}")
                    nc.tensor.matmul(lg[:], wlog[a], ts_h[h][:],
                                     start=True, stop=True)
                    nc.scalar.activation(dst[x][:, S(s)], lg[:], Act.Exp,
                                         bias=ebias)
            pt_sb = dst["k"]
            qe_sb = dst["q"]

            # ---- A = cumsum(P) + eps; qp = probsQ / A ----
            a_sb = sp.tile([128, SEQ * T_LEN], F32, name="A")
            ra_sb = sp.tile([128, SEQ * T_LEN], F32, name="ra")
            qp_sb = sp.tile([128, SEQ * T_LEN], BF, name="qp")
            for s in range(SEQ):
                _eng(nc, CFG["scan"]).tensor_tensor_scan(
                    a_sb[:, S(s)], pt_sb[:, S(s)], pt_sb[:, S(s)], EPS,
                    Alu.add, Alu.bypass)
                _eng(nc, CFG["recip"]).reciprocal_approx_fast(
                    ra_sb[:, S(s)], a_sb[:, S(s)])
                _eng(nc, CFG["qp"]).tensor_mul(
                    qp_sb[:, S(s)], qe_sb[:, S(s)], ra_sb[:, S(s)])

            # ---- chunked attention ----
            mask_b = mask1.unsqueeze(1).broadcast_to([128, SEQ, CH])
            gm_sb, pn_sb, s_sb = {}, {}, {}
            for c in range(NCH):
                gt = pw.tile([CH, SEQ * CH], F32, tag="w", name=f"gt{c}")
                for s in range(SEQ):
                    nc.tensor.matmul(gt[:, CH * s:CH * (s + 1)],
                                     pt_sb[:, tsl(s, c)], qp_sb[:, tsl(s, c)],
                                     start=True, stop=True)
                gm_sb[c] = sp.tile([CH, SEQ * CH], BF, name=f"gm{c}")
                _eng(nc, CFG["gm"][c]).tensor_mul(
                    gm_sb[c][:].rearrange("p (s t) -> p s t", s=SEQ),
                    gt[:].rearrange("p (s t) -> p s t", s=SEQ), mask_b)

                if c < NCH - 1:
                    tr = pw.tile([CH, SEQ * CH], BF, tag="w", name=f"tr{c}")
                    for s in range(SEQ):
                        nc.tensor.transpose(tr[:, CH * s:CH * (s + 1)],
                                            pt_sb[:, tsl(s, c)], ident)
                    pn_sb[c] = sp.tile([CH, SEQ * CH], BF, name=f"pn{c}")
                    _eng(nc, CFG["pn"][c]).tensor_copy(pn_sb[c][:], tr[:])
                    ds = pst.tile([LR, SEQ * D_K], F32, tag="ds", name=f"ds{c}")
                    for s in range(SEQ):
                        nc.tensor.matmul(ds[:, D_K * s:D_K * (s + 1)],
                                         pn_sb[c][:, CH * s:CH * (s + 1)],
                                         v_sb[:, vsl(s, c)],
                                         start=True, stop=True)
                    s_sb[c] = sp.tile([LR, SEQ * D_K], BF, name=f"st{c}")
                    if c == 0:
                        _eng(nc, CFG["scopy"]).tensor_copy(s_sb[c][:], ds[:])
                    else:
                        _eng(nc, CFG["scopy"]).tensor_add(
                            s_sb[c][:], ds[:], s_sb[c - 1][:])

            for c in range(NCH):
                out_ps = pw.tile([D_K, SEQ * CH], F32, tag="w", name=f"o{c}")
                for s in range(SEQ):
                    nc.tensor.matmul(out_ps[:, CH * s:CH * (s + 1)],
                                     v_sb[:, vsl(s, c)],
                                     gm_sb[c][:, CH * s:CH * (s + 1)],
                                     start=True, stop=(c == 0))
                    if c > 0:
                        nc.tensor.matmul(out_ps[:, CH * s:CH * (s + 1)],
                                         s_sb[c - 1][:, D_K * s:D_K * (s + 1)],
                                         qp_sb[:, tsl(s, c)],
                                         start=False, stop=True)
                out_sb = sp.tile([D_K, SEQ * CH], F32, name=f"osb{c}")
                nc.scalar.copy(out_sb[:], out_ps[:])
                nc.sync.dma_start(
                    out_d[:, :, CH * c:CH * (c + 1)].rearrange("s d t -> d s t"),
                    out_sb[:].rearrange("d (s t) -> d s t", s=SEQ))

            if dbg:
                for nm, src in (("P", pt_sb), ("Qe", qe_sb), ("A", a_sb),
                                ("qp", qp_sb), ("gm0", gm_sb[0]),
                                ("gm1", gm_sb[1]), ("pn0", pn_sb[0]),
                                ("st0", s_sb[0]), ("st1", s_sb[1])):
                    nc.sync.dma_start(dbg_d[nm], src[:])

    nc.compile()
    return nc


def _consts():
    """[Wproj | Wlog0 | Wlog1 | ident | mask1] as one [128, 640] bf16 blob.

    Built from the (deterministic) reference protos; planes arrive at
    runtime, so Wproj gets patched in _host_prep.
    """
    import itertools
    protos_T = np.array(
        list(itertools.product([-1.0, 1.0], repeat=K_BITS)),
        dtype=np.float32).T                               # [K, R]
    wlog = [np.zeros((128, 128), np.float32) for _ in range(2)]
    for a in range(2):
        for l in range(L_TABLES):
            r0, r1 = 16 * l, 16 * l + 16
            wlog[a][32 * a + 4 * l:32 * a + 4 * l + 4, r0:r1] = protos_T / 8.0
            wlog[a][64 + 32 * a + 4 * l:64 + 32 * a + 4 * l + 4, r0:r1] = -1.0 / 128
    ident = np.eye(128, dtype=np.float32)
    mask1 = (np.arange(CH)[:, None] <= np.arange(CH)[None, :]).astype(np.float32)
    return wlog, ident, mask1


def _host_prep(Khf, Vhf, Qhf, planes_T, protos_T):
    """Fold + transpose + quantize inputs; build per-core in_maps."""
    Khf = np.asarray(Khf, dtype=np.float32)
    Vhf = np.asarray(Vhf, dtype=np.float32)
    Qhf = np.asarray(Qhf, dtype=np.float32)
    planes_T = np.asarray(planes_T, dtype=np.float32)   # [dk, L*K]
    protos_T = np.asarray(protos_T, dtype=np.float32)   # [K, R]

    def fold(x):
        return np.transpose(x, (0, 1, 3, 2, 4)).reshape(N_TOTAL, T_LEN, D_K)

    K2, Q2, V2 = fold(Khf), fold(Qhf), fold(Vhf)
    KT = np.transpose(K2, (0, 2, 1))                    # [N, dk, T]
    QT = np.transpose(Q2, (0, 2, 1))
    V4 = V2.reshape(N_TOTAL, NCH, CH, D_K)

    wproj = np.zeros((128, 128), np.float32)
    for a in range(2):
        for b in range(2):
            wproj[64 * a:64 * a + 64, 64 * b + 32 * a:64 * b + 32 * a + 32] = planes_T

    wlog_ref, ident, mask1 = _consts()
    # rebuild wlog from the runtime protos (normally identical to reference)
    wlog = [np.zeros((128, 128), np.float32) for _ in range(2)]
    for a in range(2):
        for l in range(L_TABLES):
            r0, r1 = 16 * l, 16 * l + 16
            wlog[a][32 * a + 4 * l:32 * a + 4 * l + 4, r0:r1] = protos_T / 8.0
            wlog[a][64 + 32 * a + 4 * l:64 + 32 * a + 4 * l + 4, r0:r1] = -1.0 / 128
    cw = np.concatenate([wproj, wlog[0], wlog[1], ident, mask1],
                        axis=1).astype(BF16)            # [128, 640]
    bias_u16 = np.full((128, 1), NLOG2x4, dtype="<f4").view("<u2").reshape(128, 2)
    cw = np.concatenate([cw.view(np.uint16), bias_u16], axis=1).view(BF16)

    def pack2(arr):                                     # [4, 64, 512] -> [128, 1024]
        return np.ascontiguousarray(
            arr.reshape(2, 2, D_K, T_LEN).transpose(1, 2, 0, 3).reshape(128, 2 * T_LEN)
        ).astype(BF16)

    in_maps = []
    for core in range(NCORES):
        ns = slice(SEQ * core, SEQ * (core + 1))
        vc = np.ascontiguousarray(
            np.transpose(V4[ns], (2, 0, 1, 3))).astype(BF16)  # [128, seq, ch, dk]
        in_maps.append({
            "kt2": pack2(KT[ns]),
            "qt2": pack2(QT[ns]),
            "v": vc.reshape(CH, SEQ * NCH * D_K),
            "cw": cw,
        })
    return in_maps


def kernel(Khf, Vhf, Qhf, planes_T, protos_T, _results_hook=None):
    if "nc" not in _CACHE:
        _CACHE["nc"] = _build_module()
    nc = _CACHE["nc"]
    in_maps = _host_prep(Khf, Vhf, Qhf, planes_T, protos_T)
    res = run_bass_kernel_spmd(nc, in_maps, list(range(NCORES)))
    if _results_hook is not None:
        _results_hook(res)
    out = np.empty((N_TOTAL, T_LEN, D_K), dtype=np.float32)
    for core in range(NCORES):
        out_t = res.results[core]["out_t"]          # [SEQ, dk, T]
        out[SEQ * core:SEQ * (core + 1)] = np.transpose(out_t, (0, 2, 1))
    return np.ascontiguousarray(
        out.reshape(M_ENS, B_SZ, H_HEADS, T_LEN, D_K).transpose(0, 1, 3, 2, 4))


# revision 6
# speedup vs baseline: 1.6261x; 1.1256x over previous
"""Trainium2 Bass kernel for BatchedACE (LSH-softmax linear attention), v2.

Math (per fused sequence n of N = M*B*H = 32):
  probs(X)[t, l, r] = softmax_r( tanh(X @ planes)/sqrt(dk) @ protos )
  A = cumsum_t(probsK) + eps                [T, L, R]
  S_t = cumsum_t(probsK x V outer)          [L, R, dk]
  out[t] = sum_{l,r} probsQ[t,l,r] * S_t[l,r,:] / A[t,l,r]

v2 key trick: the per-table softmax over the 2^K hypercube corners has an
ANALYTIC partition function:
  Z[l,t] = prod_k 2*cosh(tanh_k/8)  =>  logZ = 4*log2 + sum_k tanh_k^2/128
(+O(tau^4/49152) ~ 8e-5 rel).  So probs = exp(logits - logZ) needs NO
softmax-denominator machinery: the tau^2 sum rides as extra contraction rows
in the logits matmul, and -4log2 folds into the exp bias.

Layout: seq-pair contraction packing: kt2 [128=(a,dk), 1024=(h,t)] lets one
proj matmul compute 2 sequences at once AND duplicate tanh rows into
partitions 64-127 (extra stationary columns are free), where they are
squared in place to feed the logit matmul's 128-deep contraction.

Sharding: N=32 sequences, 4 per core across 8 NeuronCores; no cross-core
communication.
"""
import math
import numpy as np
import ml_dtypes
from contextlib import ExitStack

import concourse.bass as bass
import concourse.tile as tile
from concourse import bacc, mybir
from concourse.bass_utils import run_bass_kernel_spmd

BF16 = ml_dtypes.bfloat16
BF = mybir.dt.bfloat16
F32 = mybir.dt.float32
Alu = mybir.AluOpType
Act = mybir.ActivationFunctionType

M_ENS, B_SZ, T_LEN, H_HEADS, D_K = 2, 2, 512, 8, 64
K_BITS, L_TABLES, R_CORNERS = 4, 8, 16
N_TOTAL = M_ENS * B_SZ * H_HEADS          # 32
NCORES = 8
SEQ = N_TOTAL // NCORES                   # 4 sequences per core
CH = 128                                  # chunk length (partition dim)
NCH = T_LEN // CH                         # 4 chunks
LR = L_TABLES * R_CORNERS                 # 128
LK = L_TABLES * K_BITS                    # 32
EPS = 1e-6
NLOG2x4 = -4.0 * math.log(2.0)

_CACHE = {}

# engine assignment knobs (tuned against TimelineSim)
CFG = {
    "scan": "vector",      # per-seq cumsum of probsK
    "recip": "vector",     # 1/A
    "qp": "vector",        # probsQ * ra
    "sq": "vector",        # tau^2 in place
    "gm": ["vector", "gpsimd", "vector", "gpsimd"],   # mask mul per chunk
    "pn": ["vector", "gpsimd", "vector"],             # P^T PSUM->SBUF copies
    "scopy": "vector",     # state snapshots PSUM->SBUF
}


def _eng(nc, name):
    return {"vector": nc.vector, "gpsimd": nc.gpsimd, "scalar": nc.scalar}[name]


def _copy(nc, eng, dst, src):
    if eng == "scalar":
        nc.scalar.copy(dst, src)
    else:
        _eng(nc, eng).tensor_copy(dst, src)


def _build_module(n_iters=1, dbg=False):
    """n_iters>1 wraps the body in a hardware For_i loop (timing builds)."""
    nc = bacc.Bacc("TRN2", target_bir_lowering=False, debug=False,
                   num_devices=NCORES)
    dbg_d = {}
    if dbg:
        for nm in ("P", "Qe", "A", "qp", "gm0", "gm1", "pn0", "st0", "st1"):
            w = SEQ * T_LEN if nm in ("P", "Qe", "A", "qp") else \
                (SEQ * CH if nm.startswith(("gm", "pn")) else SEQ * D_K)
            dt = F32 if nm == "A" else BF
            dbg_d[nm] = nc.dram_tensor(f"dbg_{nm}", [128, w], dt,
                                       kind="ExternalOutput").ap()

    kt2_d = nc.dram_tensor("kt2", [128, 2 * T_LEN], BF, kind="ExternalInput").ap()
    qt2_d = nc.dram_tensor("qt2", [128, 2 * T_LEN], BF, kind="ExternalInput").ap()
    v_d = nc.dram_tensor("v", [CH, SEQ * NCH * D_K], BF, kind="ExternalInput").ap()
    cw_d = nc.dram_tensor("cw", [128, 642], BF, kind="ExternalInput").ap()
    out_d = nc.dram_tensor("out_t", [SEQ, D_K, T_LEN], F32, kind="ExternalOutput").ap()

    with tile.TileContext(nc) as tc:
        with ExitStack() as ctx:
            cp = ctx.enter_context(tc.tile_pool(name="consts", bufs=1))
            sp = ctx.enter_context(tc.tile_pool(name="sb", bufs=1))
            pw = ctx.enter_context(tc.tile_pool(name="pw", bufs=6, space="PSUM"))
            pst = ctx.enter_context(tc.tile_pool(name="pst", bufs=1, space="PSUM"))
            if n_iters > 1:
                ctx.enter_context(tc.For_i(0, n_iters, 1,
                                           hint_engines=(mybir.EngineType.PE,)))

            cw_sb = cp.tile([128, 642], BF)
            nc.sync.dma_start(cw_sb[:], cw_d)
            kt2_sb = sp.tile([128, 2 * T_LEN], BF)
            nc.sync.dma_start(kt2_sb[:], kt2_d)
            qt2_sb = sp.tile([128, 2 * T_LEN], BF)
            nc.sync.dma_start(qt2_sb[:], qt2_d)
            v_sb = sp.tile([CH, SEQ * NCH * D_K], BF)
            nc.sync.dma_start(v_sb[:], v_d)

            wproj = cw_sb[:, 0:128]
            wlog = [cw_sb[:, 128:256], cw_sb[:, 256:384]]
            ident = cw_sb[:, 384:512]
            mask1 = cw_sb[:, 512:640]
            ebias = cw_sb[:, 640:642].bitcast(F32)      # [128, 1] = -4*log2

            def S(s):
                return slice(T_LEN * s, T_LEN * (s + 1))

            def tsl(s, c):
                return slice(T_LEN * s + CH * c, T_LEN * s + CH * (c + 1))

            def vsl(s, c):
                return slice(D_K * (s * NCH + c), D_K * (s * NCH + c + 1))

            # ---- probs stage: P = probsK, Qe = probsQ (both normalized) ----
            xt = {"k": kt2_sb, "q": qt2_sb}
            dst = {"k": sp.tile([128, SEQ * T_LEN], BF, name="pk"),
                   "q": sp.tile([128, SEQ * T_LEN], BF, name="pq")}

            for x in ("k", "q"):
                ts_h = []
                for h in range(2):
                    pp = pw.tile([128, T_LEN], F32, tag="w", name=f"pp{x}{h}")
                    nc.tensor.matmul(pp[:], wproj, xt[x][:, S(h)],
                                     start=True, stop=True)
                    ts = sp.tile([128, T_LEN], BF, name=f"ts{x}{h}")
                    nc.scalar.activation(ts[:], pp[:], Act.Tanh)
                    _eng(nc, CFG["sq"]).tensor_mul(
                        ts[64:128, :], ts[64:128, :], ts[64:128, :])
                    ts_h.append(ts)
                for s in range(SEQ):
                    h, a = s // 2, s % 2
                    lg = pw.tile([128, T_LEN], F32, tag="w", name=f"lg{x}{s}")
                    nc.tensor.matmul(lg[:], wlog[a], ts_h[h][:],
                                     start=True, stop=True)
                    nc.scalar.activation(dst[x][:, S(s)], lg[:], Act.Exp,
                                         bias=ebias)
            pt_sb = dst["k"]
            qe_sb = dst["q"]

            # ---- A = cumsum(P) + eps; qp = probsQ / A ----
            a_sb = sp.tile([128, SEQ * T_LEN], F32, name="A")
            ra_sb = sp.tile([128, SEQ * T_LEN], F32, name="ra")
            qp_sb = sp.tile([128, SEQ * T_LEN], BF, name="qp")
            for s in range(SEQ):
                _eng(nc, CFG["scan"]).tensor_tensor_scan(
                    a_sb[:, S(s)], pt_sb[:, S(s)], pt_sb[:, S(s)], EPS,
                    Alu.add, Alu.bypass)
                _eng(nc, CFG["recip"]).reciprocal_approx_fast(
                    ra_sb[:, S(s)], a_sb[:, S(s)])
            for s in range(SEQ):
                _eng(nc, CFG["qp"]).tensor_mul(
                    qp_sb[:, S(s)], qe_sb[:, S(s)], ra_sb[:, S(s)])

            # ---- chunked attention ----
            mask_b = mask1.unsqueeze(1).broadcast_to([128, SEQ, CH])
            gm_sb, pn_sb, s_sb = {}, {}, {}
            for c in range(NCH):
                gt = pw.tile([CH, SEQ * CH], F32, tag="w", name=f"gt{c}")
                for s in range(SEQ):
                    nc.tensor.matmul(gt[:, CH * s:CH * (s + 1)],
                                     pt_sb[:, tsl(s, c)], qp_sb[:, tsl(s, c)],
                                     start=True, stop=True)
                gm_sb[c] = sp.tile([CH, SEQ * CH], BF, name=f"gm{c}")
                _eng(nc, CFG["gm"][c]).tensor_mul(
                    gm_sb[c][:].rearrange("p (s t) -> p s t", s=SEQ),
                    gt[:].rearrange("p (s t) -> p s t", s=SEQ), mask_b)

                if c < NCH - 1:
                    tr = pw.tile([CH, SEQ * CH], BF, tag="w", name=f"tr{c}")
                    for s in range(SEQ):
                        nc.tensor.transpose(tr[:, CH * s:CH * (s + 1)],
                                            pt_sb[:, tsl(s, c)], ident)
                    pn_sb[c] = sp.tile([CH, SEQ * CH], BF, name=f"pn{c}")
                    _copy(nc, CFG["pn"][c], pn_sb[c][:], tr[:])
                    ds = pst.tile([LR, SEQ * D_K], F32, tag="ds", name=f"ds{c}")
                    for s in range(SEQ):
                        nc.tensor.matmul(ds[:, D_K * s:D_K * (s + 1)],
                                         pn_sb[c][:, CH * s:CH * (s + 1)],
                                         v_sb[:, vsl(s, c)],
                                         start=True, stop=True)
                    s_sb[c] = sp.tile([LR, SEQ * D_K], BF, name=f"st{c}")
                    _eng(nc, CFG["scopy"]).tensor_copy(s_sb[c][:], ds[:])

            for c in range(NCH):
                out_ps = pw.tile([D_K, SEQ * CH], F32, tag="w", name=f"o{c}")
                for s in range(SEQ):
                    nc.tensor.matmul(out_ps[:, CH * s:CH * (s + 1)],
                                     v_sb[:, vsl(s, c)],
                                     gm_sb[c][:, CH * s:CH * (s + 1)],
                                     start=True, stop=(c == 0))
                    for cp2 in range(c):
                        nc.tensor.matmul(out_ps[:, CH * s:CH * (s + 1)],
                                         s_sb[cp2][:, D_K * s:D_K * (s + 1)],
                                         qp_sb[:, tsl(s, c)],
                                         start=False, stop=(cp2 == c - 1))
                out_sb = sp.tile([D_K, SEQ * CH], F32, name=f"osb{c}")
                nc.scalar.copy(out_sb[:], out_ps[:])
                nc.sync.dma_start(
                    out_d[:, :, CH * c:CH * (c + 1)].rearrange("s d t -> d s t"),
                    out_sb[:].rearrange("d (s t) -> d s t", s=SEQ))

            if dbg:
                for nm, src in (("P", pt_sb), ("Qe", qe_sb), ("A", a_sb),
                                ("qp", qp_sb), ("gm0", gm_sb[0]),
                                ("gm1", gm_sb[1]), ("pn0", pn_sb[0]),
                                ("st0", s_sb[0]), ("st1", s_sb[1])):
                    nc.sync.dma_start(dbg_d[nm], src[:])

    nc.compile()
    return nc


def _consts():
    """[Wproj | Wlog0 | Wlog1 | ident | mask1] as one [128, 640] bf16 blob.

    Built from the (deterministic) reference protos; planes arrive at
    runtime, so Wproj gets patched in _host_prep.
    """
    import itertools
    protos_T = np.array(
        list(itertools.product([-1.0, 1.0], repeat=K_BITS)),
        dtype=np.float32).T                               # [K, R]
    wlog = [np.zeros((128, 128), np.float32) for _ in range(2)]
    for a in range(2):
        for l in range(L_TABLES):
            r0, r1 = 16 * l, 16 * l + 16
            wlog[a][32 * a + 4 * l:32 * a + 4 * l + 4, r0:r1] = protos_T / 8.0
            wlog[a][64 + 32 * a + 4 * l:64 + 32 * a + 4 * l + 4, r0:r1] = -1.0 / 128
    ident = np.eye(128, dtype=np.float32)
    mask1 = (np.arange(CH)[:, None] <= np.arange(CH)[None, :]).astype(np.float32)
    return wlog, ident, mask1


def _host_prep(Khf, Vhf, Qhf, planes_T, protos_T):
    """Fold + transpose + quantize inputs; build per-core in_maps."""
    Khf = np.asarray(Khf, dtype=np.float32)
    Vhf = np.asarray(Vhf, dtype=np.float32)
    Qhf = np.asarray(Qhf, dtype=np.float32)
    planes_T = np.asarray(planes_T, dtype=np.float32)   # [dk, L*K]
    protos_T = np.asarray(protos_T, dtype=np.float32)   # [K, R]

    def fold(x):
        return np.transpose(x, (0, 1, 3, 2, 4)).reshape(N_TOTAL, T_LEN, D_K)

    K2, Q2, V2 = fold(Khf), fold(Qhf), fold(Vhf)
    KT = np.transpose(K2, (0, 2, 1))                    # [N, dk, T]
    QT = np.transpose(Q2, (0, 2, 1))
    V4 = V2.reshape(N_TOTAL, NCH, CH, D_K)

    wproj = np.zeros((128, 128), np.float32)
    for a in range(2):
        for b in range(2):
            wproj[64 * a:64 * a + 64, 64 * b + 32 * a:64 * b + 32 * a + 32] = planes_T

    wlog_ref, ident, mask1 = _consts()
    # rebuild wlog from the runtime protos (normally identical to reference)
    wlog = [np.zeros((128, 128), np.float32) for _ in range(2)]
    for a in range(2):
        for l in range(L_TABLES):
            r0, r1 = 16 * l, 16 * l + 16
            wlog[a][32 * a + 4 * l:32 * a + 4 * l + 4, r0:r1] = protos_T / 8.0
            wlog[a][64 + 32 * a + 4 * l:64 + 32 * a + 4 * l + 4, r0:r1] = -1.0 / 128
    cw = np.concatenate([wproj, wlog[0], wlog[1], ident, mask1],
                        axis=1).astype(BF16)            # [128, 640]
    bias_u16 = np.full((128, 1), NLOG2x4, dtype="<f4").view("<u2").reshape(128, 2)
    cw = np.concatenate([cw.view(np.uint16), bias_u16], axis=1).view(BF16)

    def pack2(arr):                                     # [4, 64, 512] -> [128, 1024]
        return np.ascontiguousarray(
            arr.reshape(2, 2, D_K, T_LEN).transpose(1, 2, 0, 3).reshape(128, 2 * T_LEN)
        ).astype(BF16)

    in_maps = []
    for core in range(NCORES):
        ns = slice(SEQ * core, SEQ * (core + 1))
        vc = np.ascontiguousarray(
            np.transpose(V4[ns], (2, 0, 1, 3))).astype(BF16)  # [128, seq, ch, dk]
        in_maps.append({
            "kt2": pack2(KT[ns]),
            "qt2": pack2(QT[ns]),
            "v": vc.reshape(CH, SEQ * NCH * D_K),
            "cw": cw,
        })
    return in_maps


def kernel(Khf, Vhf, Qhf, planes_T, protos_T, _results_hook=None):
    if "nc" not in _CACHE:
        _CACHE["nc"] = _build_module()
    nc = _CACHE["nc"]
    in_maps = _host_prep(Khf, Vhf, Qhf, planes_T, protos_T)
    res = run_bass_kernel_spmd(nc, in_maps, list(range(NCORES)))
    if _results_hook is not None:
        _results_hook(res)
    out = np.empty((N_TOTAL, T_LEN, D_K), dtype=np.float32)
    for core in range(NCORES):
        out_t = res.results[core]["out_t"]          # [SEQ, dk, T]
        out[SEQ * core:SEQ * (core + 1)] = np.transpose(out_t, (0, 2, 1))
    return np.ascontiguousarray(
        out.reshape(M_ENS, B_SZ, H_HEADS, T_LEN, D_K).transpose(0, 1, 3, 2, 4))
